# revision 8
# baseline (speedup 1.0000x reference)
"""LocalFrameAttentionWithDiffuser on 8 TRN2 NeuronCores.

Sharding: head-parallel. Each core computes 2 of the 16 heads end-to-end
(QKV projection for its 128 hd-dims, chunked local attention, partial
output projection Y_c = O_c @ Wo[c-slice]); the host sums the 8 partial
Y tensors (bias bo is fed only to core 0 so the sum adds it once).

Shapes (hardcoded from the problem):
  x [1,16,256,1024] -> tokens T=4096, D=1024, H=16 heads, HD=64,
  chunks C=4 of L=1024 tokens; chunk i attends to chunks {i-1, i}
  (chunk 0 only to itself).

Device layout notes:
  - everything flows transposed: X^T [D, T] is a host-prepared input so
    projections produce Q^T/K^T [hd, T] directly (hd on partitions).
  - S^T = K^T.T @ Q^T per (chunk, head) with ctx on partitions, so the
    softmax sum over ctx is computed by appending a ones-column to V in
    the AV matmul (row 64 of the AV PSUM accumulates sum(exp(s))).
  - chunk 0's missing previous chunk is handled by simply not issuing
    those ctx tiles (exactly reproduces the -inf mask).
  - matmuls use float32r (full-rate fp32 path on the PE).
"""

import os
from contextlib import ExitStack

import numpy as np

import concourse.bass as bass
import concourse.tile as tile
from concourse import bacc, mybir
from concourse.bass_utils import run_bass_kernel_spmd

F32 = mybir.dt.float32
F32R = mybir.dt.float32r

B, F, N, D = 1, 16, 256, 1024
H, HD = 16, 64
CS = 4
C = F // CS            # 4 chunks
L = CS * N             # 1024 tokens per chunk
T = F * N              # 4096 tokens
NCORES = 8
HPC = H // NCORES      # 2 heads per core
HDB = HPC * HD         # 128 hd dims per core
SCALE = 1.0 / np.sqrt(HD)

TOK_TILE = 512         # moving-dim tile (fp32 max)
NDT = D // 128         # 8 contraction tiles for projections
NJT = T // TOK_TILE    # 8 token tiles
NCT = T // 128         # 32 ctx tiles of 128


def _r(ap):
    return ap.bitcast(F32R)


def build_kernel(nc, tc, outs, ins, ctx):
    xt, wq, wk, wv, wo, bo, ident = (
        ins["xt"], ins["wq"], ins["wk"], ins["wv"], ins["wo"], ins["bo"],
        ins["ident"],
    )
    y = outs["y"]

    # persistent pools: bufs=1, every tile gets a distinct name (= its own slot)
    wpool = ctx.enter_context(tc.tile_pool(name="weights", bufs=1))
    qk_pool = ctx.enter_context(tc.tile_pool(name="qk", bufs=1))
    v_pool = ctx.enter_context(tc.tile_pool(name="v", bufs=1))
    ot_pool = ctx.enter_context(tc.tile_pool(name="ot", bufs=1))
    ybias_pool = ctx.enter_context(tc.tile_pool(name="ybias", bufs=1))
    # cycling pools: shared tag -> bufs slots
    xpool = ctx.enter_context(tc.tile_pool(name="xt", bufs=6))
    vstage_pool = ctx.enter_context(tc.tile_pool(name="vstage", bufs=2))
    a_pool = ctx.enter_context(tc.tile_pool(name="attn", bufs=4))
    sum_pool = ctx.enter_context(tc.tile_pool(name="sums", bufs=4))
    bc_pool = ctx.enter_context(tc.tile_pool(name="bcast", bufs=3))
    yout_pool = ctx.enter_context(tc.tile_pool(name="yout", bufs=3))
    proj_ps = ctx.enter_context(tc.tile_pool(name="proj_ps", bufs=3, space="PSUM"))
    vtr_ps = ctx.enter_context(tc.tile_pool(name="vtr_ps", bufs=1, space="PSUM"))
    s_ps = ctx.enter_context(tc.tile_pool(name="s_ps", bufs=2, space="PSUM"))
    o_ps = ctx.enter_context(tc.tile_pool(name="o_ps", bufs=1, space="PSUM"))
    y_ps = ctx.enter_context(tc.tile_pool(name="y_ps", bufs=1, space="PSUM"))

    # ---- persistent weights / constants ----
    wq_sb = [wpool.tile([128, HDB], F32R, name=f"wq{d}") for d in range(NDT)]
    wk_sb = [wpool.tile([128, HDB], F32R, name=f"wk{d}") for d in range(NDT)]
    wv_sb = [wpool.tile([128, HDB], F32R, name=f"wv{d}") for d in range(NDT)]
    for d in range(NDT):
        nc.sync.dma_start(wq_sb[d][:], wq[d * 128:(d + 1) * 128, :].bitcast(F32R))
        nc.sync.dma_start(wk_sb[d][:], wk[d * 128:(d + 1) * 128, :].bitcast(F32R))
        nc.sync.dma_start(wv_sb[d][:], wv[d * 128:(d + 1) * 128, :].bitcast(F32R))
    wo_sb = wpool.tile([128, D], F32R, tag="wo")
    nc.sync.dma_start(wo_sb[:], wo[:, :].bitcast(F32R))
    id_sb = wpool.tile([128, 128], F32, tag="id")
    nc.sync.dma_start(id_sb[:], ident[:, :])
    ones_col = wpool.tile([128, 1], F32, tag="ones")
    nc.vector.memset(ones_col[:], 1.0)
    bo_bc = ybias_pool.tile([128, D], F32)
    nc.sync.dma_start(bo_bc[:], bo[0:1, :].broadcast_to([128, D]))

    # persistent activations
    qt_sb = qk_pool.tile([128, T], F32R, tag="qt")   # Q^T (2 heads stacked)
    kt_sb = qk_pool.tile([128, T], F32R, tag="kt")   # K^T
    ot_sb = ot_pool.tile([128, T], F32R)             # O^T normalized
    # V per ctx tile: [128 tok, 65] (64 hd + ones column), per head
    v_sb = [[v_pool.tile([128, HD + 1], F32R, name=f"v{h}_{ct}") for ct in range(NCT)]
            for h in range(HPC)]

    # ---- phase 1: projections (per 512-token tile) ----
    for j in range(NJT):
        tok = bass.ts(j, TOK_TILE)
        xt_t = [xpool.tile([128, TOK_TILE], F32R, tag="x", name=f"xt{j}_{d}") for d in range(NDT)]
        for d in range(NDT):
            nc.sync.dma_start(xt_t[d][:], xt[d * 128:(d + 1) * 128, tok].bitcast(F32R))
        q_ps = proj_ps.tile([128, TOK_TILE], F32, tag="p", name=f"qps{j}")
        k_ps = proj_ps.tile([128, TOK_TILE], F32, tag="p", name=f"kps{j}")
        vt_ps = proj_ps.tile([128, TOK_TILE], F32, tag="p", name=f"vps{j}")
        for d in range(NDT):
            st, sp = d == 0, d == NDT - 1
            nc.tensor.matmul(q_ps[:], wq_sb[d][:], xt_t[d][:], start=st, stop=sp)
            nc.tensor.matmul(k_ps[:], wk_sb[d][:], xt_t[d][:], start=st, stop=sp)
            nc.tensor.matmul(vt_ps[:], wv_sb[d][:], xt_t[d][:], start=st, stop=sp)
        nc.scalar.copy(qt_sb[:, tok], q_ps[:])
        nc.scalar.copy(kt_sb[:, tok], k_ps[:])
        vt_stage = vstage_pool.tile([128, TOK_TILE], F32, tag="vs", name=f"vst{j}")
        nc.vector.tensor_copy(vt_stage[:], vt_ps[:])
        # transpose V^T -> V in 128x128 blocks; split the two heads
        for kblk in range(TOK_TILE // 128):
            ct = j * (TOK_TILE // 128) + kblk
            vtr = vtr_ps.tile([128, 128], F32, tag="vt", name=f"vtr{j}_{kblk}")
            nc.tensor.transpose(vtr[:], vt_stage[:, bass.ts(kblk, 128)], id_sb[:])
            for h in range(HPC):
                nc.vector.tensor_copy(v_sb[h][ct][:, 0:HD], vtr[:, h * HD:(h + 1) * HD])
                nc.vector.tensor_copy(v_sb[h][ct][:, HD:HD + 1], ones_col[:])

    # ---- phase 2+3: attention per chunk, then its slice of the output proj ----
    for c in range(C):
        cts = list(range(max(0, 8 * (c - 1)), 8 * (c + 1)))  # ctx tiles (128 tok)
        for th in range(L // TOK_TILE):  # 2 query halves per chunk
            tok0 = c * L + th * TOK_TILE
            tok = bass.ds(tok0, TOK_TILE)
            for h in range(HPC):
                hr = slice(h * HD, (h + 1) * HD)
                o_acc = o_ps.tile([HD + 1, TOK_TILE], F32, tag="o", name=f"ops{c}_{th}_{h}")
                for ci, ct in enumerate(cts):
                    s_t = s_ps.tile([128, TOK_TILE], F32, tag="s", name=f"sps{c}_{th}_{h}_{ci}")
                    nc.tensor.matmul(
                        s_t[:], kt_sb[hr, bass.ts(ct, 128)], qt_sb[hr, tok],
                        start=True, stop=True,
                    )
                    a_t = a_pool.tile([128, TOK_TILE], F32R, tag="a", name=f"a{c}_{th}_{h}_{ci}")
                    nc.scalar.activation(
                        a_t[:], s_t[:], mybir.ActivationFunctionType.Exp, scale=SCALE
                    )
                    nc.tensor.matmul(
                        o_acc[:], v_sb[h][ct][:], a_t[:],
                        start=(ci == 0), stop=(ci == len(cts) - 1),
                    )
                # normalize: rows 0:64 / row 64
                s_sum = sum_pool.tile([1, TOK_TILE], F32, tag="s", name=f"ssum{c}_{th}_{h}")
                nc.vector.reciprocal(s_sum[:], o_acc[HD:HD + 1, :])
                r_bc = bc_pool.tile([HD, TOK_TILE], F32, tag="bc", name=f"bc{c}_{th}_{h}")
                nc.gpsimd.partition_broadcast(r_bc[:], s_sum[0:1, :])
                nc.vector.tensor_mul(ot_sb[hr, tok], o_acc[0:HD, :], r_bc[:])
        # output projection for this chunk's 8 token tiles
        for m in range(8 * c, 8 * (c + 1)):
            for dh in range(D // TOK_TILE):
                yp = y_ps.tile([128, TOK_TILE], F32, tag="y", name=f"yps{m}_{dh}")
                nc.tensor.matmul(
                    yp[:], ot_sb[:, bass.ts(m, 128)],
                    wo_sb[:, bass.ts(dh, TOK_TILE)],
                    start=True, stop=True,
                )
                y_sb = yout_pool.tile([128, TOK_TILE], F32, tag="yo", name=f"yo{m}_{dh}")
                nc.vector.tensor_add(y_sb[:], yp[:], bo_bc[:, bass.ts(dh, TOK_TILE)])
                nc.sync.dma_start(y[bass.ts(m, 128), bass.ts(dh, TOK_TILE)], y_sb[:])


_CACHE = {}


def _build():
    if "nc" in _CACHE:
        return _CACHE["nc"]
    nc = bacc.Bacc(
        "TRN2",
        target_bir_lowering=False,
        debug=False,
        enable_asserts=False,
        num_devices=NCORES,
    )
    ins = {
        "xt": nc.dram_tensor("xt", [D, T], F32, kind="ExternalInput").ap(),
        "wq": nc.dram_tensor("wq", [D, HDB], F32, kind="ExternalInput").ap(),
        "wk": nc.dram_tensor("wk", [D, HDB], F32, kind="ExternalInput").ap(),
        "wv": nc.dram_tensor("wv", [D, HDB], F32, kind="ExternalInput").ap(),
        "wo": nc.dram_tensor("wo", [HDB, D], F32, kind="ExternalInput").ap(),
        "bo": nc.dram_tensor("bo", [1, D], F32, kind="ExternalInput").ap(),
        "ident": nc.dram_tensor("ident", [128, 128], F32, kind="ExternalInput").ap(),
    }
    outs = {"y": nc.dram_tensor("y", [T, D], F32, kind="ExternalOutput").ap()}
    with tile.TileContext(nc, trace_sim=False) as tc:
        with ExitStack() as kctx:
            build_kernel(nc, tc, outs, ins, kctx)
    nc.compile()
    _CACHE["nc"] = nc
    return nc


def make_in_maps(x, Wq, Wk, Wv, Wo, bo):
    xt = np.ascontiguousarray(
        np.asarray(x, dtype=np.float32).reshape(T, D).T
    )
    ident = np.eye(128, dtype=np.float32)
    bo = np.asarray(bo, dtype=np.float32).reshape(1, D)
    zeros_bo = np.zeros_like(bo)
    in_maps = []
    for core in range(NCORES):
        hs = slice(core * HDB, (core + 1) * HDB)
        in_maps.append({
            "xt": xt,
            "wq": np.ascontiguousarray(np.asarray(Wq, np.float32)[:, hs]),
            "wk": np.ascontiguousarray(np.asarray(Wk, np.float32)[:, hs]),
            "wv": np.ascontiguousarray(np.asarray(Wv, np.float32)[:, hs]),
            "wo": np.ascontiguousarray(np.asarray(Wo, np.float32)[hs, :]),
            "bo": bo if core == 0 else zeros_bo,
            "ident": ident,
        })
    return in_maps


def kernel(x, Wq, Wk, Wv, Wo, bo, _trace=False, _tmpdir=None):
    nc = _build()
    in_maps = make_in_maps(x, Wq, Wk, Wv, Wo, bo)
    res = run_bass_kernel_spmd(
        nc, in_maps, core_ids=list(range(NCORES)),
        trace=_trace, tmpdir=_tmpdir,
        **({"trace_cores": list(range(NCORES))} if _trace else {}),
    )
    if _trace:
        kernel.last_results = res
    y = np.zeros((T, D), dtype=np.float32)
    for r in res.results:
        y += r["y"]
    return y.reshape(B, F, N, D)


# revision 14
# speedup vs baseline: 163.5816x; 163.5816x over previous
"""LocalFrameAttentionWithDiffuser on 8 TRN2 NeuronCores.

Sharding: head-parallel. Each core computes 2 of the 16 heads end-to-end
(QKV projection for its 128 hd-dims, chunked local attention, partial
output projection Y_c = O_c @ Wo[c-slice]); the host sums the 8 partial
Y tensors (bias bo is fed only to core 0 so the sum adds it once).

Shapes (hardcoded from the problem):
  x [1,16,256,1024] -> tokens T=4096, D=1024, H=16 heads, HD=64,
  chunks C=4 of L=1024 tokens; chunk i attends to chunks {i-1, i}
  (chunk 0 only to itself).

Device layout notes:
  - everything flows transposed: X^T [D, T] is a host-prepared input so
    projections produce Q^T/K^T [hd, T] directly (hd on partitions).
  - S^T = K^T.T @ Q^T per (chunk, head) with ctx on partitions, so the
    softmax sum over ctx is computed by appending a ones-column to V in
    the AV matmul (row 64 of the AV PSUM accumulates sum(exp(s))).
  - chunk 0's missing previous chunk is handled by simply not issuing
    those ctx tiles (exactly reproduces the -inf mask).
  - matmuls use float32r (full-rate fp32 path on the PE).
"""

import os
from contextlib import ExitStack

import numpy as np

import concourse.bass as bass
import concourse.tile as tile
from concourse import bacc, mybir
from concourse.bass_utils import run_bass_kernel_spmd

F32 = mybir.dt.float32
F32R = mybir.dt.float32r

B, F, N, D = 1, 16, 256, 1024
H, HD = 16, 64
CS = 4
C = F // CS            # 4 chunks
L = CS * N             # 1024 tokens per chunk
T = F * N              # 4096 tokens
NCORES = 8
HPC = H // NCORES      # 2 heads per core
HDB = HPC * HD         # 128 hd dims per core
SCALE = 1.0 / np.sqrt(HD)

TOK_TILE = 512         # moving-dim tile (fp32 max)
NDT = D // 128         # 8 contraction tiles for projections
NJT = T // TOK_TILE    # 8 token tiles
NCT = T // 128         # 32 ctx tiles of 128


def _r(ap):
    return ap.bitcast(F32R)


def build_kernel(nc, tc, outs, ins, ctx, phases=3):
    xt, wq, wk, wv, wo, bo, ident = (
        ins["xt"], ins["wq"], ins["wk"], ins["wv"], ins["wo"], ins["bo"],
        ins["ident"],
    )
    y = outs["y"]

    # persistent pools: bufs=1, every tile gets a distinct name (= its own slot)
    wpool = ctx.enter_context(tc.tile_pool(name="weights", bufs=1))
    qk_pool = ctx.enter_context(tc.tile_pool(name="qk", bufs=1))
    v_pool = ctx.enter_context(tc.tile_pool(name="v", bufs=1))
    ot_pool = ctx.enter_context(tc.tile_pool(name="ot", bufs=1))
    ybias_pool = ctx.enter_context(tc.tile_pool(name="ybias", bufs=1))
    # cycling pools: shared tag -> bufs slots
    xpool = ctx.enter_context(tc.tile_pool(name="xt", bufs=16))
    vstage_pool = ctx.enter_context(tc.tile_pool(name="vstage", bufs=2))
    a_pool = ctx.enter_context(tc.tile_pool(name="attn", bufs=8))
    sum_pool = ctx.enter_context(tc.tile_pool(name="sums", bufs=4))
    bc_pool = ctx.enter_context(tc.tile_pool(name="bcast", bufs=3))
    yout_pool = ctx.enter_context(tc.tile_pool(name="yout", bufs=3))
    ps_pool = ctx.enter_context(tc.tile_pool(name="ps", bufs=8, space="PSUM"))
    proj_ps = vtr_ps = s_ps = o_ps = y_ps = ps_pool

    # ---- persistent weights / constants (loaded at first use) ----
    wq_sb = [wpool.tile([128, HDB], F32R, name=f"wq{d}") for d in range(NDT)]
    wk_sb = [wpool.tile([128, HDB], F32R, name=f"wk{d}") for d in range(NDT)]
    wv_sb = [wpool.tile([128, HDB], F32R, name=f"wv{d}") for d in range(NDT)]
    wo_sb = wpool.tile([128, D], F32R, tag="wo")
    id_sb = wpool.tile([128, 128], F32, tag="id")
    ones_col = wpool.tile([128, 1], F32, tag="ones")
    nc.vector.memset(ones_col[:], 1.0)
    bo_bc = ybias_pool.tile([128, D], F32)

    # persistent activations
    qt_sb = qk_pool.tile([128, T], F32R, tag="qt")   # Q^T (2 heads stacked)
    kt_sb = qk_pool.tile([128, T], F32R, tag="kt")   # K^T
    ot_sb = ot_pool.tile([128, T], F32R)             # O^T normalized
    # V per ctx tile: [128 tok, 65] (64 hd + ones column), per head
    v_sb = [[v_pool.tile([128, HD + 1], F32R, name=f"v{h}_{ct}") for ct in range(NCT)]
            for h in range(HPC)]

    # ---- phase 1: projections (per 512-token tile) ----
    for j in range(NJT):
        tok = bass.ts(j, TOK_TILE)
        xt_t = [xpool.tile([128, TOK_TILE], F32R, tag="x", name=f"xt{j}_{d}") for d in range(NDT)]
        for d in range(NDT):
            nc.sync.dma_start(xt_t[d][:], xt[d * 128:(d + 1) * 128, tok].bitcast(F32R))
            if j == 0:
                nc.sync.dma_start(wq_sb[d][:], wq[d * 128:(d + 1) * 128, :].bitcast(F32R))
                nc.sync.dma_start(wk_sb[d][:], wk[d * 128:(d + 1) * 128, :].bitcast(F32R))
                nc.sync.dma_start(wv_sb[d][:], wv[d * 128:(d + 1) * 128, :].bitcast(F32R))
        if j == 0:
            nc.sync.dma_start(id_sb[:], ident[:, :])
        if j == 2:
            nc.sync.dma_start(wo_sb[:], wo[:, :].bitcast(F32R))
            nc.sync.dma_start(bo_bc[:], bo[0:1, :].broadcast_to([128, D]))
        q_ps = proj_ps.tile([128, TOK_TILE], F32, tag="ps", name=f"qps{j}")
        k_ps = proj_ps.tile([128, TOK_TILE], F32, tag="ps", name=f"kps{j}")
        vt_ps = proj_ps.tile([128, TOK_TILE], F32, tag="ps", name=f"vps{j}")
        for d in range(NDT):
            st, sp = d == 0, d == NDT - 1
            nc.tensor.matmul(q_ps[:], wq_sb[d][:], xt_t[d][:], start=st, stop=sp)
            nc.tensor.matmul(k_ps[:], wk_sb[d][:], xt_t[d][:], start=st, stop=sp)
            nc.tensor.matmul(vt_ps[:], wv_sb[d][:], xt_t[d][:], start=st, stop=sp)
        nc.vector.tensor_copy(qt_sb[:, tok], q_ps[:])
        nc.vector.tensor_copy(kt_sb[:, tok], k_ps[:])
        vt_stage = vstage_pool.tile([128, TOK_TILE], F32, tag="vs", name=f"vst{j}")
        nc.vector.tensor_copy(vt_stage[:], vt_ps[:])
        # transpose V^T -> V in 128x128 blocks; split the two heads
        for kblk in range(TOK_TILE // 128):
            ct = j * (TOK_TILE // 128) + kblk
            vtr = vtr_ps.tile([128, 128], F32, tag="ps", name=f"vtr{j}_{kblk}")
            nc.tensor.transpose(vtr[:], vt_stage[:, bass.ts(kblk, 128)], id_sb[:])
            for h in range(HPC):
                nc.vector.tensor_copy(v_sb[h][ct][:, 0:HD], vtr[:, h * HD:(h + 1) * HD])
                nc.vector.tensor_copy(v_sb[h][ct][:, HD:HD + 1], ones_col[:])

    # ---- phase 2+3: attention per chunk, then its slice of the output proj ----
    if phases < 2:
        return
    for c in range(C):
        cts = list(range(max(0, 8 * (c - 1)), 8 * (c + 1)))  # ctx tiles (128 tok)
        for th in range(L // TOK_TILE):  # 2 query halves per chunk
            tok0 = c * L + th * TOK_TILE
            tok = bass.ds(tok0, TOK_TILE)
            for h in range(HPC):
                hr = slice(h * HD, (h + 1) * HD)
                o_acc = o_ps.tile([HD + 1, TOK_TILE], F32, tag="ps", name=f"ops{c}_{th}_{h}")
                for ci, ct in enumerate(cts):
                    s_t = s_ps.tile([128, TOK_TILE], F32, tag="ps", name=f"sps{c}_{th}_{h}_{ci}")
                    nc.tensor.matmul(
                        s_t[:], kt_sb[hr, bass.ts(ct, 128)], qt_sb[hr, tok],
                        start=True, stop=True,
                    )
                    a_t = a_pool.tile([128, TOK_TILE], F32R, tag="a", name=f"a{c}_{th}_{h}_{ci}")
                    nc.scalar.activation(
                        a_t[:], s_t[:], mybir.ActivationFunctionType.Exp, scale=SCALE
                    )
                    nc.tensor.matmul(
                        o_acc[:], v_sb[h][ct][:], a_t[:],
                        start=(ci == 0), stop=(ci == len(cts) - 1),
                    )
                # normalize: rows 0:64 / row 64
                s_sum = sum_pool.tile([1, TOK_TILE], F32, tag="s", name=f"ssum{c}_{th}_{h}")
                nc.vector.reciprocal(s_sum[:], o_acc[HD:HD + 1, :])
                r_bc = bc_pool.tile([HD, TOK_TILE], F32, tag="bc", name=f"bc{c}_{th}_{h}")
                nc.gpsimd.partition_broadcast(r_bc[:], s_sum[0:1, :])
                nc.vector.tensor_mul(ot_sb[hr, tok], o_acc[0:HD, :], r_bc[:])
            # output projection for this half-chunk's 4 token tiles
            for m in ([] if phases < 3 else range(8 * c + 4 * th, 8 * c + 4 * (th + 1))):
                for dh in range(D // TOK_TILE):
                    yp = y_ps.tile([128, TOK_TILE], F32, tag="ps", name=f"yps{m}_{dh}")
                    nc.tensor.matmul(
                        yp[:], ot_sb[:, bass.ts(m, 128)],
                        wo_sb[:, bass.ts(dh, TOK_TILE)],
                        start=True, stop=True,
                    )
                    y_sb = yout_pool.tile([128, TOK_TILE], F32, tag="yo", name=f"yo{m}_{dh}")
                    nc.vector.tensor_add(y_sb[:], yp[:], bo_bc[:, bass.ts(dh, TOK_TILE)])
                    nc.sync.dma_start(y[bass.ts(m, 128), bass.ts(dh, TOK_TILE)], y_sb[:])


_CACHE = {}


def _build(phases=3):
    if ("nc", phases) in _CACHE:
        return _CACHE[("nc", phases)]
    nc = bacc.Bacc(
        "TRN2",
        target_bir_lowering=False,
        debug=False,
        enable_asserts=False,
        num_devices=NCORES,
    )
    ins = {
        "xt": nc.dram_tensor("xt", [D, T], F32, kind="ExternalInput").ap(),
        "wq": nc.dram_tensor("wq", [D, HDB], F32, kind="ExternalInput").ap(),
        "wk": nc.dram_tensor("wk", [D, HDB], F32, kind="ExternalInput").ap(),
        "wv": nc.dram_tensor("wv", [D, HDB], F32, kind="ExternalInput").ap(),
        "wo": nc.dram_tensor("wo", [HDB, D], F32, kind="ExternalInput").ap(),
        "bo": nc.dram_tensor("bo", [1, D], F32, kind="ExternalInput").ap(),
        "ident": nc.dram_tensor("ident", [128, 128], F32, kind="ExternalInput").ap(),
    }
    outs = {"y": nc.dram_tensor("y", [T, D], F32, kind="ExternalOutput").ap()}
    with tile.TileContext(nc, trace_sim=False) as tc:
        with ExitStack() as kctx:
            build_kernel(nc, tc, outs, ins, kctx, phases=phases)
    nc.compile()
    _CACHE[("nc", phases)] = nc
    return nc


def make_in_maps(x, Wq, Wk, Wv, Wo, bo):
    xt = np.ascontiguousarray(
        np.asarray(x, dtype=np.float32).reshape(T, D).T
    )
    ident = np.eye(128, dtype=np.float32)
    bo = np.asarray(bo, dtype=np.float32).reshape(1, D)
    zeros_bo = np.zeros_like(bo)
    in_maps = []
    for core in range(NCORES):
        hs = slice(core * HDB, (core + 1) * HDB)
        in_maps.append({
            "xt": xt,
            "wq": np.ascontiguousarray(np.asarray(Wq, np.float32)[:, hs]),
            "wk": np.ascontiguousarray(np.asarray(Wk, np.float32)[:, hs]),
            "wv": np.ascontiguousarray(np.asarray(Wv, np.float32)[:, hs]),
            "wo": np.ascontiguousarray(np.asarray(Wo, np.float32)[hs, :]),
            "bo": bo if core == 0 else zeros_bo,
            "ident": ident,
        })
    return in_maps


def kernel(x, Wq, Wk, Wv, Wo, bo, _trace=False, _tmpdir=None):
    nc = _build()
    in_maps = make_in_maps(x, Wq, Wk, Wv, Wo, bo)
    res = run_bass_kernel_spmd(
        nc, in_maps, core_ids=list(range(NCORES)),
        trace=_trace, tmpdir=_tmpdir,
        **({"trace_cores": list(range(NCORES))} if _trace else {}),
    )
    if _trace:
        kernel.last_results = res
    y = np.zeros((T, D), dtype=np.float32)
    for r in res.results:
        y += r["y"]
    return y.reshape(B, F, N, D)


# revision 15
# speedup vs baseline: 166.2667x; 1.0164x over previous
"""LocalFrameAttentionWithDiffuser on 8 TRN2 NeuronCores.

Sharding: head-parallel. Each core computes 2 of the 16 heads end-to-end
(QKV projection for its 128 hd-dims, chunked local attention, partial
output projection Y_c = O_c @ Wo[c-slice]); the host sums the 8 partial
Y tensors (bias bo is fed only to core 0 so the sum adds it once).

Shapes (hardcoded from the problem):
  x [1,16,256,1024] -> tokens T=4096, D=1024, H=16 heads, HD=64,
  chunks C=4 of L=1024 tokens; chunk i attends to chunks {i-1, i}
  (chunk 0 only to itself).

Device layout notes:
  - everything flows transposed: X^T [D, T] is a host-prepared input so
    projections produce Q^T/K^T [hd, T] directly (hd on partitions).
  - S^T = K^T.T @ Q^T per (chunk, head) with ctx on partitions, so the
    softmax sum over ctx is computed by appending a ones-column to V in
    the AV matmul (row 64 of the AV PSUM accumulates sum(exp(s))).
  - chunk 0's missing previous chunk is handled by simply not issuing
    those ctx tiles (exactly reproduces the -inf mask).
  - matmuls use float32r (full-rate fp32 path on the PE).
"""

import os
from contextlib import ExitStack

import numpy as np

import concourse.bass as bass
import concourse.tile as tile
from concourse import bacc, mybir
from concourse.bass_utils import run_bass_kernel_spmd

F32 = mybir.dt.float32
F32R = mybir.dt.float32r

B, F, N, D = 1, 16, 256, 1024
H, HD = 16, 64
CS = 4
C = F // CS            # 4 chunks
L = CS * N             # 1024 tokens per chunk
T = F * N              # 4096 tokens
NCORES = 8
HPC = H // NCORES      # 2 heads per core
HDB = HPC * HD         # 128 hd dims per core
SCALE = 1.0 / np.sqrt(HD)

TOK_TILE = 512         # moving-dim tile (fp32 max)
NDT = D // 128         # 8 contraction tiles for projections
NJT = T // TOK_TILE    # 8 token tiles
NCT = T // 128         # 32 ctx tiles of 128


def _r(ap):
    return ap.bitcast(F32R)


def build_kernel(nc, tc, outs, ins, ctx, phases=3):
    xt, wq, wk, wv, wo, bo, ident = (
        ins["xt"], ins["wq"], ins["wk"], ins["wv"], ins["wo"], ins["bo"],
        ins["ident"],
    )
    y = outs["y"]

    # persistent pools: bufs=1, every tile gets a distinct name (= its own slot)
    wpool = ctx.enter_context(tc.tile_pool(name="weights", bufs=1))
    qk_pool = ctx.enter_context(tc.tile_pool(name="qk", bufs=1))
    v_pool = ctx.enter_context(tc.tile_pool(name="v", bufs=1))
    ot_pool = ctx.enter_context(tc.tile_pool(name="ot", bufs=1))
    ybias_pool = ctx.enter_context(tc.tile_pool(name="ybias", bufs=1))
    # cycling pools: shared tag -> bufs slots
    xpool = ctx.enter_context(tc.tile_pool(name="xt", bufs=16))
    vstage_pool = ctx.enter_context(tc.tile_pool(name="vstage", bufs=2))
    a_pool = ctx.enter_context(tc.tile_pool(name="attn", bufs=8))
    sum_pool = ctx.enter_context(tc.tile_pool(name="sums", bufs=4))
    bc_pool = ctx.enter_context(tc.tile_pool(name="bcast", bufs=4))
    yout_pool = ctx.enter_context(tc.tile_pool(name="yout", bufs=6))
    ps_pool = ctx.enter_context(tc.tile_pool(name="ps", bufs=8, space="PSUM"))
    proj_ps = vtr_ps = s_ps = o_ps = y_ps = ps_pool

    # ---- persistent weights / constants (loaded at first use) ----
    wq_sb = [wpool.tile([128, HDB], F32R, name=f"wq{d}") for d in range(NDT)]
    wk_sb = [wpool.tile([128, HDB], F32R, name=f"wk{d}") for d in range(NDT)]
    wv_sb = [wpool.tile([128, HDB], F32R, name=f"wv{d}") for d in range(NDT)]
    wo_sb = wpool.tile([128, D], F32R, tag="wo")
    id_sb = wpool.tile([128, 128], F32, tag="id")
    ones_col = wpool.tile([128, 1], F32, tag="ones")
    nc.vector.memset(ones_col[:], 1.0)
    bo_bc = ybias_pool.tile([128, D], F32)

    # persistent activations
    qt_sb = qk_pool.tile([128, T], F32R, tag="qt")   # Q^T (2 heads stacked)
    kt_sb = qk_pool.tile([128, T], F32R, tag="kt")   # K^T
    ot_sb = ot_pool.tile([128, T], F32R)             # O^T normalized
    # V per ctx tile: [128 tok, 65] (64 hd + ones column), per head
    v_sb = [[v_pool.tile([128, HD + 1], F32R, name=f"v{h}_{ct}") for ct in range(NCT)]
            for h in range(HPC)]

    # ---- phase 1: projections (per 512-token tile) ----
    for j in range(NJT):
        tok = bass.ts(j, TOK_TILE)
        xt_t = [xpool.tile([128, TOK_TILE], F32R, tag="x", name=f"xt{j}_{d}") for d in range(NDT)]
        for d in range(NDT):
            nc.sync.dma_start(xt_t[d][:], xt[d * 128:(d + 1) * 128, tok].bitcast(F32R))
            if j == 0:
                nc.sync.dma_start(wq_sb[d][:], wq[d * 128:(d + 1) * 128, :].bitcast(F32R))
                nc.sync.dma_start(wk_sb[d][:], wk[d * 128:(d + 1) * 128, :].bitcast(F32R))
                nc.sync.dma_start(wv_sb[d][:], wv[d * 128:(d + 1) * 128, :].bitcast(F32R))
        if j == 0:
            nc.sync.dma_start(id_sb[:], ident[:, :])
        if j == 2:
            nc.sync.dma_start(wo_sb[:], wo[:, :].bitcast(F32R))
            nc.sync.dma_start(bo_bc[:], bo[0:1, :].broadcast_to([128, D]))
        q_ps = proj_ps.tile([128, TOK_TILE], F32, tag="ps", name=f"qps{j}")
        k_ps = proj_ps.tile([128, TOK_TILE], F32, tag="ps", name=f"kps{j}")
        vt_ps = proj_ps.tile([128, TOK_TILE], F32, tag="ps", name=f"vps{j}")
        for d in range(NDT):
            st, sp = d == 0, d == NDT - 1
            nc.tensor.matmul(q_ps[:], wq_sb[d][:], xt_t[d][:], start=st, stop=sp)
            nc.tensor.matmul(k_ps[:], wk_sb[d][:], xt_t[d][:], start=st, stop=sp)
            nc.tensor.matmul(vt_ps[:], wv_sb[d][:], xt_t[d][:], start=st, stop=sp)
        nc.vector.tensor_copy(qt_sb[:, tok], q_ps[:])
        nc.vector.tensor_copy(kt_sb[:, tok], k_ps[:])
        vt_stage = vstage_pool.tile([128, TOK_TILE], F32, tag="vs", name=f"vst{j}")
        nc.vector.tensor_copy(vt_stage[:], vt_ps[:])
        # transpose V^T -> V in 128x128 blocks; split the two heads
        for kblk in range(TOK_TILE // 128):
            ct = j * (TOK_TILE // 128) + kblk
            vtr = vtr_ps.tile([128, 128], F32, tag="ps", name=f"vtr{j}_{kblk}")
            nc.tensor.transpose(vtr[:], vt_stage[:, bass.ts(kblk, 128)], id_sb[:])
            for h in range(HPC):
                nc.vector.tensor_copy(v_sb[h][ct][:, 0:HD], vtr[:, h * HD:(h + 1) * HD])
                nc.vector.tensor_copy(v_sb[h][ct][:, HD:HD + 1], ones_col[:])

    # ---- phase 2+3: attention per chunk, then its slice of the output proj ----
    if phases < 2:
        return
    for c in range(C):
        cts = list(range(max(0, 8 * (c - 1)), 8 * (c + 1)))  # ctx tiles (128 tok)
        for th in range(L // TOK_TILE):  # 2 query halves per chunk
            tok0 = c * L + th * TOK_TILE
            tok = bass.ds(tok0, TOK_TILE)
            for h in range(HPC):
                hr = slice(h * HD, (h + 1) * HD)
                o_acc = o_ps.tile([HD + 1, TOK_TILE], F32, tag="ps", name=f"ops{c}_{th}_{h}")
                for ci, ct in enumerate(cts):
                    s_t = s_ps.tile([128, TOK_TILE], F32, tag="ps", name=f"sps{c}_{th}_{h}_{ci}")
                    nc.tensor.matmul(
                        s_t[:], kt_sb[hr, bass.ts(ct, 128)], qt_sb[hr, tok],
                        start=True, stop=True,
                    )
                    a_t = a_pool.tile([128, TOK_TILE], F32R, tag="a", name=f"a{c}_{th}_{h}_{ci}")
                    nc.scalar.activation(
                        a_t[:], s_t[:], mybir.ActivationFunctionType.Exp, scale=SCALE
                    )
                    nc.tensor.matmul(
                        o_acc[:], v_sb[h][ct][:], a_t[:],
                        start=(ci == 0), stop=(ci == len(cts) - 1),
                    )
                # normalize: rows 0:64 / row 64
                s_sum = sum_pool.tile([1, TOK_TILE], F32, tag="s", name=f"ssum{c}_{th}_{h}")
                nc.vector.reciprocal(s_sum[:], o_acc[HD:HD + 1, :])
                r_bc = bc_pool.tile([HD, TOK_TILE], F32, tag="bc", name=f"bc{c}_{th}_{h}")
                nc.gpsimd.partition_broadcast(r_bc[:], s_sum[0:1, :])
                nc.vector.tensor_mul(ot_sb[hr, tok], o_acc[0:HD, :], r_bc[:])
            # output projection for this half-chunk's 4 token tiles
            for m in ([] if phases < 3 else range(8 * c + 4 * th, 8 * c + 4 * (th + 1))):
                for dh in range(D // TOK_TILE):
                    yp = y_ps.tile([128, TOK_TILE], F32, tag="ps", name=f"yps{m}_{dh}")
                    nc.tensor.matmul(
                        yp[:], ot_sb[:, bass.ts(m, 128)],
                        wo_sb[:, bass.ts(dh, TOK_TILE)],
                        start=True, stop=True,
                    )
                    y_sb = yout_pool.tile([128, TOK_TILE], F32, tag="yo", name=f"yo{m}_{dh}")
                    nc.vector.tensor_add(y_sb[:], yp[:], bo_bc[:, bass.ts(dh, TOK_TILE)])
                    nc.sync.dma_start(y[bass.ts(m, 128), bass.ts(dh, TOK_TILE)], y_sb[:])


_CACHE = {}


def _build(phases=3):
    if ("nc", phases) in _CACHE:
        return _CACHE[("nc", phases)]
    nc = bacc.Bacc(
        "TRN2",
        target_bir_lowering=False,
        debug=False,
        enable_asserts=False,
        num_devices=NCORES,
    )
    ins = {
        "xt": nc.dram_tensor("xt", [D, T], F32, kind="ExternalInput").ap(),
        "wq": nc.dram_tensor("wq", [D, HDB], F32, kind="ExternalInput").ap(),
        "wk": nc.dram_tensor("wk", [D, HDB], F32, kind="ExternalInput").ap(),
        "wv": nc.dram_tensor("wv", [D, HDB], F32, kind="ExternalInput").ap(),
        "wo": nc.dram_tensor("wo", [HDB, D], F32, kind="ExternalInput").ap(),
        "bo": nc.dram_tensor("bo", [1, D], F32, kind="ExternalInput").ap(),
        "ident": nc.dram_tensor("ident", [128, 128], F32, kind="ExternalInput").ap(),
    }
    outs = {"y": nc.dram_tensor("y", [T, D], F32, kind="ExternalOutput").ap()}
    with tile.TileContext(nc, trace_sim=False) as tc:
        with ExitStack() as kctx:
            build_kernel(nc, tc, outs, ins, kctx, phases=phases)
    nc.compile()
    _CACHE[("nc", phases)] = nc
    return nc


def make_in_maps(x, Wq, Wk, Wv, Wo, bo):
    xt = np.ascontiguousarray(
        np.asarray(x, dtype=np.float32).reshape(T, D).T
    )
    ident = np.eye(128, dtype=np.float32)
    bo = np.asarray(bo, dtype=np.float32).reshape(1, D)
    zeros_bo = np.zeros_like(bo)
    in_maps = []
    for core in range(NCORES):
        hs = slice(core * HDB, (core + 1) * HDB)
        in_maps.append({
            "xt": xt,
            "wq": np.ascontiguousarray(np.asarray(Wq, np.float32)[:, hs]),
            "wk": np.ascontiguousarray(np.asarray(Wk, np.float32)[:, hs]),
            "wv": np.ascontiguousarray(np.asarray(Wv, np.float32)[:, hs]),
            "wo": np.ascontiguousarray(np.asarray(Wo, np.float32)[hs, :]),
            "bo": bo if core == 0 else zeros_bo,
            "ident": ident,
        })
    return in_maps


def kernel(x, Wq, Wk, Wv, Wo, bo, _trace=False, _tmpdir=None):
    nc = _build()
    in_maps = make_in_maps(x, Wq, Wk, Wv, Wo, bo)
    res = run_bass_kernel_spmd(
        nc, in_maps, core_ids=list(range(NCORES)),
        trace=_trace, tmpdir=_tmpdir,
        **({"trace_cores": list(range(NCORES))} if _trace else {}),
    )
    if _trace:
        kernel.last_results = res
    y = np.zeros((T, D), dtype=np.float32)
    for r in res.results:
        y += r["y"]
    return y.reshape(B, F, N, D)


# revision 18
# speedup vs baseline: 166.2836x; 1.0001x over previous
"""LocalFrameAttentionWithDiffuser on 8 TRN2 NeuronCores.

Sharding: head-parallel. Each core computes 2 of the 16 heads end-to-end
(QKV projection for its 128 hd-dims, chunked local attention, partial
output projection Y_c = O_c @ Wo[c-slice]); the host sums the 8 partial
Y tensors (bias bo is fed only to core 0 so the sum adds it once).

Shapes (hardcoded from the problem):
  x [1,16,256,1024] -> tokens T=4096, D=1024, H=16 heads, HD=64,
  chunks C=4 of L=1024 tokens; chunk i attends to chunks {i-1, i}
  (chunk 0 only to itself).

Device layout notes:
  - everything flows transposed: X^T [D, T] is a host-prepared input so
    projections produce Q^T/K^T [hd, T] directly (hd on partitions).
  - S^T = K^T.T @ Q^T per (chunk, head) with ctx on partitions, so the
    softmax sum over ctx is computed by appending a ones-column to V in
    the AV matmul (row 64 of the AV PSUM accumulates sum(exp(s))).
  - chunk 0's missing previous chunk is handled by simply not issuing
    those ctx tiles (exactly reproduces the -inf mask).
  - matmuls use float32r (full-rate fp32 path on the PE).
"""

import os
from contextlib import ExitStack

import numpy as np

import concourse.bass as bass
import concourse.tile as tile
from concourse import bacc, mybir
from concourse.bass_utils import run_bass_kernel_spmd

F32 = mybir.dt.float32
F32R = mybir.dt.float32r

B, F, N, D = 1, 16, 256, 1024
H, HD = 16, 64
CS = 4
C = F // CS            # 4 chunks
L = CS * N             # 1024 tokens per chunk
T = F * N              # 4096 tokens
NCORES = 8
HPC = H // NCORES      # 2 heads per core
HDB = HPC * HD         # 128 hd dims per core
SCALE = 1.0 / np.sqrt(HD)

TOK_TILE = 512         # moving-dim tile (fp32 max)
NDT = D // 128         # 8 contraction tiles for projections
NJT = T // TOK_TILE    # 8 token tiles
NCT = T // 128         # 32 ctx tiles of 128


def _r(ap):
    return ap.bitcast(F32R)


def build_kernel(nc, tc, outs, ins, ctx, phases=3):
    xt, wq, wk, wv, wo, bo, ident = (
        ins["xt"], ins["wq"], ins["wk"], ins["wv"], ins["wo"], ins["bo"],
        ins["ident"],
    )
    y = outs["y"]

    # persistent pools: bufs=1, every tile gets a distinct name (= its own slot)
    wpool = ctx.enter_context(tc.tile_pool(name="weights", bufs=1))
    qk_pool = ctx.enter_context(tc.tile_pool(name="qk", bufs=1))
    v_pool = ctx.enter_context(tc.tile_pool(name="v", bufs=1))
    ot_pool = ctx.enter_context(tc.tile_pool(name="ot", bufs=1))
    ybias_pool = ctx.enter_context(tc.tile_pool(name="ybias", bufs=1))
    # cycling pools: shared tag -> bufs slots
    xpool = ctx.enter_context(tc.tile_pool(name="xt", bufs=16))
    vstage_pool = ctx.enter_context(tc.tile_pool(name="vstage", bufs=2))
    a_pool = ctx.enter_context(tc.tile_pool(name="attn", bufs=12))
    sum_pool = ctx.enter_context(tc.tile_pool(name="sums", bufs=8))
    bc_pool = ctx.enter_context(tc.tile_pool(name="bcast", bufs=4))
    yout_pool = ctx.enter_context(tc.tile_pool(name="yout", bufs=6))
    ps_pool = ctx.enter_context(tc.tile_pool(name="ps", bufs=8, space="PSUM"))
    proj_ps = vtr_ps = s_ps = o_ps = y_ps = ps_pool

    # ---- persistent weights / constants (loaded at first use) ----
    wq_sb = [wpool.tile([128, HDB], F32R, name=f"wq{d}") for d in range(NDT)]
    wk_sb = [wpool.tile([128, HDB], F32R, name=f"wk{d}") for d in range(NDT)]
    wv_sb = [wpool.tile([128, HDB], F32R, name=f"wv{d}") for d in range(NDT)]
    wo_sb = wpool.tile([128, D], F32R, tag="wo")
    id_sb = wpool.tile([128, 128], F32, tag="id")
    ones_col = wpool.tile([128, 1], F32, tag="ones")
    nc.vector.memset(ones_col[:], 1.0)
    bo_bc = ybias_pool.tile([128, D], F32)

    # persistent activations
    qt_sb = qk_pool.tile([128, T], F32R, tag="qt")   # Q^T (2 heads stacked)
    kt_sb = qk_pool.tile([128, T], F32R, tag="kt")   # K^T
    ot_sb = ot_pool.tile([128, T], F32R)             # O^T normalized
    # V per ctx tile: [128 tok, 65] (64 hd + ones column), per head
    v_sb = [[v_pool.tile([128, HD + 1], F32R, name=f"v{h}_{ct}") for ct in range(NCT)]
            for h in range(HPC)]

    # ---- phase 1: projections (per 512-token tile) ----
    for j in range(NJT):
        tok = bass.ts(j, TOK_TILE)
        xt_t = [xpool.tile([128, TOK_TILE], F32R, tag="x", name=f"xt{j}_{d}") for d in range(NDT)]
        for d in range(NDT):
            nc.sync.dma_start(xt_t[d][:], xt[d * 128:(d + 1) * 128, tok].bitcast(F32R))
            if j == 0:
                nc.sync.dma_start(wq_sb[d][:], wq[d * 128:(d + 1) * 128, :].bitcast(F32R))
                nc.sync.dma_start(wk_sb[d][:], wk[d * 128:(d + 1) * 128, :].bitcast(F32R))
                nc.sync.dma_start(wv_sb[d][:], wv[d * 128:(d + 1) * 128, :].bitcast(F32R))
        if j == 0:
            nc.sync.dma_start(id_sb[:], ident[:, :])
        if j == 2:
            nc.sync.dma_start(wo_sb[:], wo[:, :].bitcast(F32R))
            nc.sync.dma_start(bo_bc[:], bo[0:1, :].broadcast_to([128, D]))
        q_ps = proj_ps.tile([128, TOK_TILE], F32, tag="ps", name=f"qps{j}")
        k_ps = proj_ps.tile([128, TOK_TILE], F32, tag="ps", name=f"kps{j}")
        vt_ps = proj_ps.tile([128, TOK_TILE], F32, tag="ps", name=f"vps{j}")
        for d in range(NDT):
            st, sp = d == 0, d == NDT - 1
            nc.tensor.matmul(q_ps[:], wq_sb[d][:], xt_t[d][:], start=st, stop=sp)
            nc.tensor.matmul(k_ps[:], wk_sb[d][:], xt_t[d][:], start=st, stop=sp)
            nc.tensor.matmul(vt_ps[:], wv_sb[d][:], xt_t[d][:], start=st, stop=sp)
        nc.vector.tensor_copy(qt_sb[:, tok], q_ps[:])
        nc.vector.tensor_copy(kt_sb[:, tok], k_ps[:])
        vt_stage = vstage_pool.tile([128, TOK_TILE], F32, tag="vs", name=f"vst{j}")
        nc.vector.tensor_copy(vt_stage[:], vt_ps[:])
        # transpose V^T -> V in 128x128 blocks; split the two heads
        for kblk in range(TOK_TILE // 128):
            ct = j * (TOK_TILE // 128) + kblk
            vtr = vtr_ps.tile([128, 128], F32, tag="ps", name=f"vtr{j}_{kblk}")
            nc.tensor.transpose(vtr[:], vt_stage[:, bass.ts(kblk, 128)], id_sb[:])
            for h in range(HPC):
                nc.vector.tensor_copy(v_sb[h][ct][:, 0:HD], vtr[:, h * HD:(h + 1) * HD])
                nc.gpsimd.tensor_copy(v_sb[h][ct][:, HD:HD + 1], ones_col[:])

    # ---- phase 2+3: attention per chunk, then its slice of the output proj ----
    if phases < 2:
        return
    for c in range(C):
        cts = list(range(max(0, 8 * (c - 1)), 8 * (c + 1)))  # ctx tiles (128 tok)
        for th in range(L // TOK_TILE):  # 2 query halves per chunk
            tok0 = c * L + th * TOK_TILE
            tok = bass.ds(tok0, TOK_TILE)
            for h in range(HPC):
                hr = slice(h * HD, (h + 1) * HD)
                o_acc = o_ps.tile([HD + 1, TOK_TILE], F32, tag="ps", name=f"ops{c}_{th}_{h}")
                for ci, ct in enumerate(cts):
                    s_t = s_ps.tile([128, TOK_TILE], F32, tag="ps", name=f"sps{c}_{th}_{h}_{ci}")
                    nc.tensor.matmul(
                        s_t[:], kt_sb[hr, bass.ts(ct, 128)], qt_sb[hr, tok],
                        start=True, stop=True,
                    )
                    a_t = a_pool.tile([128, TOK_TILE], F32R, tag="a", name=f"a{c}_{th}_{h}_{ci}")
                    nc.scalar.activation(
                        a_t[:], s_t[:], mybir.ActivationFunctionType.Exp, scale=SCALE
                    )
                    nc.tensor.matmul(
                        o_acc[:], v_sb[h][ct][:], a_t[:],
                        start=(ci == 0), stop=(ci == len(cts) - 1),
                    )
                # normalize: rows 0:64 / row 64
                s_sum = sum_pool.tile([1, TOK_TILE], F32, tag="s", name=f"ssum{c}_{th}_{h}")
                nc.vector.reciprocal(s_sum[:], o_acc[HD:HD + 1, :])
                r_bc = bc_pool.tile([HD, TOK_TILE], F32, tag="bc", name=f"bc{c}_{th}_{h}")
                nc.gpsimd.partition_broadcast(r_bc[:], s_sum[0:1, :])
                nc.vector.tensor_mul(ot_sb[hr, tok], o_acc[0:HD, :], r_bc[:])
            # output projection for this half-chunk's 4 token tiles
            for m in ([] if phases < 3 else range(8 * c + 4 * th, 8 * c + 4 * (th + 1))):
                for dh in range(D // TOK_TILE):
                    yp = y_ps.tile([128, TOK_TILE], F32, tag="ps", name=f"yps{m}_{dh}")
                    nc.tensor.matmul(
                        yp[:], ot_sb[:, bass.ts(m, 128)],
                        wo_sb[:, bass.ts(dh, TOK_TILE)],
                        start=True, stop=True,
                    )
                    y_sb = yout_pool.tile([128, TOK_TILE], F32, tag="yo", name=f"yo{m}_{dh}")
                    nc.vector.tensor_add(y_sb[:], yp[:], bo_bc[:, bass.ts(dh, TOK_TILE)])
                    nc.sync.dma_start(y[bass.ts(m, 128), bass.ts(dh, TOK_TILE)], y_sb[:])


_CACHE = {}


def _build(phases=3):
    if ("nc", phases) in _CACHE:
        return _CACHE[("nc", phases)]
    nc = bacc.Bacc(
        "TRN2",
        target_bir_lowering=False,
        debug=False,
        enable_asserts=False,
        num_devices=NCORES,
    )
    ins = {
        "xt": nc.dram_tensor("xt", [D, T], F32, kind="ExternalInput").ap(),
        "wq": nc.dram_tensor("wq", [D, HDB], F32, kind="ExternalInput").ap(),
        "wk": nc.dram_tensor("wk", [D, HDB], F32, kind="ExternalInput").ap(),
        "wv": nc.dram_tensor("wv", [D, HDB], F32, kind="ExternalInput").ap(),
        "wo": nc.dram_tensor("wo", [HDB, D], F32, kind="ExternalInput").ap(),
        "bo": nc.dram_tensor("bo", [1, D], F32, kind="ExternalInput").ap(),
        "ident": nc.dram_tensor("ident", [128, 128], F32, kind="ExternalInput").ap(),
    }
    outs = {"y": nc.dram_tensor("y", [T, D], F32, kind="ExternalOutput").ap()}
    with tile.TileContext(nc, trace_sim=False) as tc:
        with ExitStack() as kctx:
            build_kernel(nc, tc, outs, ins, kctx, phases=phases)
    nc.compile()
    _CACHE[("nc", phases)] = nc
    return nc


def make_in_maps(x, Wq, Wk, Wv, Wo, bo):
    xt = np.ascontiguousarray(
        np.asarray(x, dtype=np.float32).reshape(T, D).T
    )
    ident = np.eye(128, dtype=np.float32)
    bo = np.asarray(bo, dtype=np.float32).reshape(1, D)
    zeros_bo = np.zeros_like(bo)
    in_maps = []
    for core in range(NCORES):
        hs = slice(core * HDB, (core + 1) * HDB)
        in_maps.append({
            "xt": xt,
            "wq": np.ascontiguousarray(np.asarray(Wq, np.float32)[:, hs]),
            "wk": np.ascontiguousarray(np.asarray(Wk, np.float32)[:, hs]),
            "wv": np.ascontiguousarray(np.asarray(Wv, np.float32)[:, hs]),
            "wo": np.ascontiguousarray(np.asarray(Wo, np.float32)[hs, :]),
            "bo": bo if core == 0 else zeros_bo,
            "ident": ident,
        })
    return in_maps


def kernel(x, Wq, Wk, Wv, Wo, bo, _trace=False, _tmpdir=None):
    nc = _build()
    in_maps = make_in_maps(x, Wq, Wk, Wv, Wo, bo)
    res = run_bass_kernel_spmd(
        nc, in_maps, core_ids=list(range(NCORES)),
        trace=_trace, tmpdir=_tmpdir,
        **({"trace_cores": list(range(NCORES))} if _trace else {}),
    )
    if _trace:
        kernel.last_results = res
    y = np.zeros((T, D), dtype=np.float32)
    for r in res.results:
        y += r["y"]
    return y.reshape(B, F, N, D)
